# revision 28
# baseline (speedup 1.0000x reference)
"""GQA attention (RoPE, causal) + o_proj on 8 Trainium2 NeuronCores.

Sharding: 8 cores = 2 batches (DP) x 4 kv-head-pairs (TP over GQA groups).
Per core: hsT[batch] [D,S] (host-pretransposed), Wq slice (8 q heads), Wk/Wv
slice (2 kv heads), Wo slice [512,D]. Core computes its heads' attention and a
partial o_proj output [S,D] (fp16); host sums 4 partials per batch in fp32.

v2 kernel dataflow (per core; fp16 matmul operands, fp32 PSUM accumulate),
J-pipelined over 4 seq-blocks of 512:
  1. qT/kT produced DIRECTLY transposed: stationary W-tile [128D x 128f],
     streaming hsT [128D x 512s] -> psum [f, s]. No PE transposes for q/k.
  2. RoPE in transposed layout: host pair-interleaves each head's features
     (p=2i <-> x_i, p=2i+1 <-> x_{i+32}) so rotate-half == partition pair-swap
     == DVE stream_shuffle(mask=[1,0,3,2,...]). Tables are per-partition
     [128, S] fp16, sign-folded, q-scaled.
  3. v transposed back to natural via 4 PE transposes per J-block -> vaug
     [seq, 65] with ones column for the softmax denominator.
  4. scoresT[k,q] per 128k-tile x 512q block, 2 heads row-packed via
     tile_position (0,0)/(64,0); causal k-tiles only; 2 k-tiles per PSUM
     [128,1024]; exp on ACT -> fp16 pt.
  5. causal mask: gpsimd affine_select zeroes the [128,128] triangle on
     diagonal tiles only; fully-masked column blocks are simply excluded
     from the AV streams (column-restricted accumulation).
  6. AT_aug = [V|1].T @ P^T accumulated over k-tiles in PSUM; row 64 is the
     denominator; normalize via reciprocal + gpsimd partition_broadcast.
  7. o_proj(J-1) quarters interleaved into attention(J) as PE filler;
     y emitted fp16.
"""
import sys
import numpy as np

sys.path.insert(0, "/opt/trn_rl_repo")

B, S, D = 2, 2048, 2048
H, KVH, HD = 32, 8, 64
SCALE = HD ** -0.5
P = 128


def build_nc(S=S, D=D, LQ=8, LKV=2, HD=64):
    import concourse.bacc as bacc
    import concourse.mybir as mybir
    from concourse import tile
    from concourse.masks import make_identity

    f32 = mybir.dt.float32
    f16 = mybir.dt.float16

    QF = LQ * HD          # local q features (512)
    KF = LKV * HD         # local kv features (128)
    FT = QF // P          # q feature tiles = head pairs (4)
    DT = D // P           # contraction tiles (16)
    ST = S // P           # sequence k-tiles (16)
    NJ = S // 512         # seq super-blocks (4)
    SB = 512              # seq block size
    VW = HD + 1           # v + ones column (65)
    Exp = mybir.ActivationFunctionType.Exp
    NEG = -30000.0

    nc = bacc.Bacc(None, target_bir_lowering=False)
    hsT = nc.declare_dram_parameter("hsT", [D, S], f16, isOutput=False)
    wq = nc.declare_dram_parameter("wq", [D, QF], f16, isOutput=False)
    wk = nc.declare_dram_parameter("wk", [D, KF], f16, isOutput=False)
    wv = nc.declare_dram_parameter("wv", [D, KF], f16, isOutput=False)
    wo = nc.declare_dram_parameter("wo", [QF, D], f16, isOutput=False)
    cq = nc.declare_dram_parameter("cq", [P, S], f16, isOutput=False)
    sq = nc.declare_dram_parameter("sq", [P, S], f16, isOutput=False)
    ck = nc.declare_dram_parameter("ck", [P, S], f16, isOutput=False)
    sk = nc.declare_dram_parameter("sk", [P, S], f16, isOutput=False)
    out = nc.declare_dram_parameter("out", [S, D], f16, isOutput=True)

    hsT_r = hsT.rearrange("(dt p) s -> p dt s", p=P)
    wq_r = wq.rearrange("(dt p) f -> p dt f", p=P)
    wk_r = wk.rearrange("(dt p) f -> p dt f", p=P)
    wv_r = wv.rearrange("(dt p) f -> p dt f", p=P)
    wo_r = wo.rearrange("(ft p) d -> p ft d", p=P)
    out_r = out.rearrange("(st p) d -> p st d", p=P)

    # stream_shuffle mask: swap adjacent pairs within each 32-lane quadrant
    pairswap = []
    for i in range(16):
        pairswap += [2 * i + 1, 2 * i]

    with tile.TileContext(nc) as tc:
        with tc.tile_pool(name="persist", bufs=1) as persist:
            ident = persist.tile([P, P], f16)
            tri = persist.tile([P, P], f16)
            k2T = persist.tile([P, LKV, S], f16)
            vaug = persist.tile([P, ST, LKV * VW], f16)
            cq_sb = persist.tile([P, S], f16)
            sq_sb = persist.tile([P, S], f16)
            ck_sb = persist.tile([P, S], f16)
            sk_sb = persist.tile([P, S], f16)
            wq_sb = persist.tile([P, DT, QF], f16)
            wk_sb = persist.tile([P, DT, KF], f16)
            wv_sb = persist.tile([P, DT, KF], f16)
            wo_sb = persist.tile([P, FT, D], f16)

            with tc.tile_pool(name="init", bufs=1) as init_p:
                ident_f = init_p.tile([P, P], f32)
                make_identity(nc, ident_f[:])
                nc.vector.tensor_copy(ident[:], ident_f[:])
                # tri[kr, j] = 1 where j >= kr else 0 (causal triangle keep-mask)
                tri_f = init_p.tile([P, P], f32)
                nc.gpsimd.memset(tri_f[:], 1.0)
                nc.gpsimd.affine_select(
                    out=tri_f[:], in_=tri_f[:],
                    compare_op=mybir.AluOpType.is_ge, fill=0.0,
                    base=0, pattern=[[1, P]], channel_multiplier=-1)
                nc.vector.tensor_copy(tri[:], tri_f[:])
                # ones columns of vaug (col 64 and 129 of the 130-wide dim)
                for lkv in range(LKV):
                    nc.gpsimd.memset(vaug[:, :, lkv * VW + HD:lkv * VW + VW], 1.0)

            with (
                tc.tile_pool(name="hs_p", bufs=2) as hs_p,
                tc.tile_pool(name="rope_p", bufs=3) as rope_p,
                tc.tile_pool(name="qT_p", bufs=2) as qT_p,
                tc.tile_pool(name="aT_p", bufs=2) as aT_p,
                tc.tile_pool(name="pt_p", bufs=18) as pt_p,
                tc.tile_pool(name="y_p", bufs=4) as y_p,
                tc.tile_pool(name="rd_p", bufs=4) as rd_p,
                tc.tile_pool(name="bc_p", bufs=4) as bc_p,
                tc.tile_pool(name="pmix", bufs=2, space="PSUM") as pmix,
                tc.tile_pool(name="ps_s", bufs=2, space="PSUM") as ps_s,
                tc.tile_pool(name="ps_a", bufs=2, space="PSUM") as ps_a,
            ):
                # DMA order matters: the first projection chain needs hsT(J0)
                # in dt order and wq's first feature tile; tables shortly
                # after; wo last. hs is split so the first chain can start
                # before the whole block lands.
                hsb_first = hs_p.tile([P, DT, SB], f16, tag="hsb")
                wq_r4 = wq.rearrange("(dt p) (ft f) -> p ft dt f", p=P, f=P)
                wq_sb4 = wq_sb[:].rearrange("p dt (ft f) -> p ft dt f", f=P)
                # two parallel DMA streams: hs/k-side on the sync engine's
                # queue, q-side weights + tables + wv/wo on gpsimd's SWDGE
                nc.sync.dma_start(
                    out=hsb_first[:, 0:4], in_=hsT_r[:, 0:4, 0:SB])
                nc.gpsimd.dma_start(out=wq_sb4[:, 0], in_=wq_r4[:, 0])
                nc.sync.dma_start(
                    out=hsb_first[:, 4:8], in_=hsT_r[:, 4:8, 0:SB])
                nc.gpsimd.dma_start(out=cq_sb[:], in_=cq[:, :])
                nc.gpsimd.dma_start(out=sq_sb[:], in_=sq[:, :])
                nc.sync.dma_start(
                    out=hsb_first[:, 8:16], in_=hsT_r[:, 8:16, 0:SB])
                nc.gpsimd.dma_start(out=wq_sb4[:, 1], in_=wq_r4[:, 1])
                nc.gpsimd.dma_start(out=wq_sb4[:, 2], in_=wq_r4[:, 2])
                nc.gpsimd.dma_start(out=wq_sb4[:, 3], in_=wq_r4[:, 3])
                nc.sync.dma_start(out=wk_sb[:], in_=wk_r)
                nc.sync.dma_start(out=ck_sb[:], in_=ck[:, :])
                nc.sync.dma_start(out=sk_sb[:], in_=sk[:, :])
                nc.gpsimd.dma_start(out=wv_sb[:], in_=wv_r)
                nc.gpsimd.dma_start(out=wo_sb[:], in_=wo_r)
                hsb_second = hs_p.tile([P, DT, SB], f16, tag="hsb")
                nc.sync.dma_start(out=hsb_second[:], in_=hsT_r[:, :, SB:2 * SB])
                def rope(ps, cos_sb, sin_sb, j0, dest):
                    """dest = ps * cos + pairswap(ps * sin_pre)  [P, SB] fp16.

                    sin_pre is the host-pre-shuffled (and sign-folded) sin
                    table, so the pair-swap happens AFTER the multiply and
                    stream_shuffle runs same-dtype fp16->fp16.
                    """
                    u = rope_p.tile([P, SB], f16, tag="u")
                    nc.vector.tensor_mul(
                        u[:], ps[:], sin_sb[:, j0:j0 + SB])
                    tmp = rope_p.tile([P, SB], f16, tag="tmp")
                    nc.vector.tensor_mul(
                        tmp[:], ps[:], cos_sb[:, j0:j0 + SB])
                    sh = rope_p.tile([P, SB], f16, tag="sh")
                    nc.vector.stream_shuffle(sh[:], u[:], mask=pairswap)
                    nc.vector.tensor_add(dest, tmp[:], sh[:])

                def o_proj_quarter(aTt, Jp, stl):
                    st = 4 * Jp + stl
                    yt = y_p.tile([P, D], f16, tag="yt")
                    for dn_i in range(D // 512):
                        psy = pmix.tile([P, 512], f32, tag="ps512")
                        for ft in range(FT):
                            nc.tensor.matmul(
                                psy[:],
                                aTt[:, ft, stl * P:(stl + 1) * P],
                                wo_sb[:, ft, dn_i * 512:(dn_i + 1) * 512],
                                start=(ft == 0), stop=(ft == FT - 1))
                        nc.any.tensor_copy(
                            yt[:, dn_i * 512:(dn_i + 1) * 512], psy[:])
                    nc.sync.dma_start(out=out_r[:, st, :], in_=yt[:])

                def proj(J, hsb):
                    """projections for seq block J -> (qT tile, k2T/vaug)."""
                    j0 = J * SB
                    qTt = qT_p.tile([P, FT, SB], f16, tag="qT")
                    for ft in range(FT):
                        psq = pmix.tile([P, SB], f32, tag="ps512")
                        for dt in range(DT):
                            nc.tensor.matmul(
                                psq[:],
                                wq_sb[:, dt, ft * P:(ft + 1) * P],
                                hsb[:, dt, :],
                                start=(dt == 0), stop=(dt == DT - 1))
                        rope(psq, cq_sb, sq_sb, j0, qTt[:, ft, :])

                    psk = pmix.tile([P, SB], f32, tag="ps512")
                    for dt in range(DT):
                        nc.tensor.matmul(
                            psk[:], wk_sb[:, dt, :], hsb[:, dt, :],
                            start=(dt == 0), stop=(dt == DT - 1))
                    krot = rope_p.tile([P, SB], f16, tag="krot")
                    rope(psk, ck_sb, sk_sb, j0, krot[:])
                    nc.vector.tensor_copy(
                        k2T[0:HD, 0, j0:j0 + SB], krot[0:HD, :])
                    nc.vector.tensor_copy(
                        k2T[HD:P, 0, j0:j0 + SB], krot[0:HD, :])
                    nc.vector.tensor_copy(
                        k2T[0:HD, 1, j0:j0 + SB], krot[HD:P, :])
                    nc.vector.tensor_copy(
                        k2T[HD:P, 1, j0:j0 + SB], krot[HD:P, :])

                    # v natural-direct (stationary hs block) -> vaug
                    psv = pmix.tile([P, SB], f32, tag="ps512")
                    for si in range(4):
                        for dt in range(DT):
                            nc.tensor.matmul(
                                psv[:, si * P:(si + 1) * P],
                                hsb[:, dt, si * P:(si + 1) * P],
                                wv_sb[:, dt, :],
                                start=(dt == 0), stop=(dt == DT - 1))
                    psv_v = psv[:].rearrange("p (q f) -> p q f", q=4)
                    nc.vector.tensor_copy(
                        vaug[:, 4 * J:4 * J + 4, 0:HD], psv_v[:, :, 0:HD])
                    nc.vector.tensor_copy(
                        vaug[:, 4 * J:4 * J + 4, VW:VW + HD], psv_v[:, :, HD:P])
                    return qTt

                # macro-order: proj0 proj1 attn0 proj2 attn1+op0 proj3
                # attn2+op1 attn3+op2 op3 -- every attention phase has
                # projection or o_proj matmuls available as PE filler
                qTs = {0: proj(0, hsb_first), 1: proj(1, hsb_second)}
                hsbs = {}
                prev_aT = None
                for J in range(NJ):
                    j0 = J * SB
                    qTt = qTs.pop(J)
                    if J + 2 < NJ:
                        hsb_pf = hs_p.tile([P, DT, SB], f16, tag="hsb")
                        hsbs[J + 2] = hsb_pf
                        nc.sync.dma_start(
                            out=hsb_pf[:],
                            in_=hsT_r[:, :, (J + 2) * SB:(J + 3) * SB])

                    # ---- attention for this J block ----
                    nkt = 4 * J + 4
                    aTt = aT_p.tile([P, FT, SB], f16, tag="aT")
                    for t in range(FT):          # head pair (2t, 2t+1)
                        lkv = t // 2
                        # scores: both heads of the pair share one PSUM tile
                        # (cols 0:512 head A, 512:1024 head B) so the two
                        # row-group matmuls become ready together and stay
                        # adjacent -> true tile_position concurrency.
                        pts = []
                        for kt in range(nkt):
                            pss = ps_s.tile([P, 1024], f32, tag="pss")
                            nc.tensor.matmul(
                                pss[:, 0:512],
                                k2T[0:HD, lkv, kt * P:(kt + 1) * P],
                                qTt[0:HD, t, :],
                                start=True, stop=True,
                                tile_position=(0, 0))
                            nc.tensor.matmul(
                                pss[:, 512:1024],
                                k2T[HD:P, lkv, kt * P:(kt + 1) * P],
                                qTt[HD:P, t, :],
                                start=True, stop=True,
                                tile_position=(HD, 0))
                            pt = pt_p.tile([P, 1024], f16, tag="pt")
                            m = kt - 4 * J
                            if m >= 1:
                                # diagonal tile: columns < 128m are fully
                                # masked; exp only the visible column ranges
                                # of both heads via a strided AP
                                pss_v = pss[:].rearrange(
                                    "p (h c) -> p h c", h=2)
                                pt_v = pt[:].rearrange("p (h c) -> p h c", h=2)
                                nc.scalar.activation(
                                    pt_v[:, :, m * P:512],
                                    pss_v[:, :, m * P:512], Exp)
                            else:
                                nc.scalar.activation(pt[:], pss[:], Exp)
                            if m >= 0:               # diagonal tile: triangle
                                nc.vector.tensor_mul(
                                    pt[:, m * P:(m + 1) * P],
                                    pt[:, m * P:(m + 1) * P], tri[:])
                                nc.vector.tensor_mul(
                                    pt[:, 512 + m * P:512 + (m + 1) * P],
                                    pt[:, 512 + m * P:512 + (m + 1) * P],
                                    tri[:])
                            pts.append(pt)
                        psa0 = ps_a.tile([VW, 512], f32, tag="psa")
                        psa1 = ps_a.tile([VW, 512], f32, tag="psa")
                        for kt in range(nkt):
                            pt = pts[kt]
                            c0 = (kt - 4 * J) * P if kt >= 4 * J else 0
                            nc.tensor.matmul(
                                psa0[:, c0:512],
                                vaug[:, kt, lkv * VW:(lkv + 1) * VW],
                                pt[:, c0:512],
                                start=(kt == 0), stop=(kt == nkt - 1),
                                skip_group_check=True)
                            nc.tensor.matmul(
                                psa1[:, c0:512],
                                vaug[:, kt, lkv * VW:(lkv + 1) * VW],
                                pt[:, 512 + c0:1024],
                                start=(kt == 0), stop=(kt == nkt - 1),
                                skip_group_check=True)
                        # normalize: psa rows 0..63 / row 64
                        for psa, poff in ((psa0, 0), (psa1, HD)):
                            dn = rd_p.tile([1, 512], f32, tag="dn")
                            nc.vector.tensor_copy(dn[:], psa[HD:VW, :])
                            rc = rd_p.tile([1, 512], f32, tag="rc")
                            nc.vector.reciprocal_approx_fast(rc[:], dn[:])
                            dnb = bc_p.tile([HD, 512], f32, tag="bc")
                            nc.gpsimd.partition_broadcast(dnb[:], rc[:])
                            nc.vector.tensor_mul(
                                aTt[poff:poff + HD, t, :], psa[0:HD, :], dnb[:])
                        # PE filler while exp/normalize run
                        if prev_aT is not None:
                            o_proj_quarter(prev_aT, J - 1, t)
                    prev_aT = aTt
                    if J + 2 < NJ:
                        qTs[J + 2] = proj(J + 2, hsbs.pop(J + 2))
                for stl in range(4):
                    o_proj_quarter(prev_aT, NJ - 1, stl)
    nc.compile()
    return nc


def _host_tables(LQ, LKV, scale):
    """Per-partition [128, S] fp16 tables in pair-interleaved feature order.

    partition p (within a 64-feature head): freq index i = p//2;
    p even holds x_i   -> rot term is -x_{i+32} * sin -> sin sign -1
    p odd  holds x_{i+32} -> rot term is +x_i * sin   -> sin sign +1
    """
    hd = HD
    inv_freq = 1.0 / (10000.0 ** (np.arange(0, hd, 2, dtype=np.float64) / hd))
    t = np.arange(S, dtype=np.float64)
    freqs = np.outer(inv_freq, t)                    # [32, S]
    cos64 = np.repeat(np.cos(freqs), 2, axis=0)      # [64, S]
    sin64 = np.repeat(np.sin(freqs), 2, axis=0)
    sgn = np.where(np.arange(64) % 2 == 0, -1.0, 1.0)[:, None]
    sin64 = sin64 * sgn
    # pre-shuffle rows by the pair-swap so the kernel can shuffle AFTER
    # multiplying (keeps stream_shuffle same-dtype fp16->fp16)
    rowswap = np.arange(64) ^ 1
    sin64 = sin64[rowswap]
    cosP = np.tile(cos64, (2, 1))                    # [128, S]
    sinP = np.tile(sin64, (2, 1))
    cq = (cosP * scale).astype(np.float16)
    sq = (sinP * scale).astype(np.float16)
    ck = cosP.astype(np.float16)
    sk = sinP.astype(np.float16)
    return cq, sq, ck, sk


def _perm64():
    """pair-interleave: new position 2i <- old i, 2i+1 <- old 32+i."""
    p = np.empty(64, dtype=np.int64)
    p[0::2] = np.arange(32)
    p[1::2] = np.arange(32) + 32
    return p


def prepare_in_maps(hidden_states, cos, sin, Wq, Wk, Wv, Wo, LQ=8, LKV=2):
    cq, sq, ck, sk = _host_tables(LQ, LKV, SCALE)
    nb = hidden_states.shape[0]
    hsT = [np.ascontiguousarray(hidden_states[b].T).astype(np.float16)
           for b in range(nb)]
    p64 = _perm64()
    permQ = np.concatenate([p64 + 64 * h for h in range(LQ)])
    permK = np.concatenate([p64 + 64 * h for h in range(LKV)])
    in_maps = []
    for c in range(8):
        b, g2 = c // 4, c % 4
        qs = g2 * LQ * HD
        ks = g2 * LKV * HD
        wq_l = Wq[:, qs:qs + LQ * HD][:, permQ]
        wk_l = Wk[:, ks:ks + LKV * HD][:, permK]
        wv_l = Wv[:, ks:ks + LKV * HD]
        in_maps.append({
            "hsT": hsT[b],
            "wq": np.ascontiguousarray(wq_l).astype(np.float16),
            "wk": np.ascontiguousarray(wk_l).astype(np.float16),
            "wv": np.ascontiguousarray(wv_l).astype(np.float16),
            "wo": np.ascontiguousarray(Wo[qs:qs + LQ * HD, :]).astype(np.float16),
            "cq": cq, "sq": sq, "ck": ck, "sk": sk,
        })
    return in_maps


_NC_CACHE = {}


def kernel(hidden_states, attention_mask, cos, sin, Wq, Wk, Wv, Wo):
    from concourse.bass_utils import run_bass_kernel_spmd

    hidden_states = np.asarray(hidden_states, dtype=np.float32)
    cos = np.asarray(cos, dtype=np.float32)
    sin = np.asarray(sin, dtype=np.float32)
    Wq = np.asarray(Wq, dtype=np.float32)
    Wk = np.asarray(Wk, dtype=np.float32)
    Wv = np.asarray(Wv, dtype=np.float32)
    Wo = np.asarray(Wo, dtype=np.float32)

    LQ, LKV = 8, 2
    if "nc" not in _NC_CACHE:
        _NC_CACHE["nc"] = build_nc(S, D, LQ, LKV, HD)
    nc = _NC_CACHE["nc"]

    in_maps = prepare_in_maps(hidden_states, cos, sin, Wq, Wk, Wv, Wo, LQ, LKV)
    res = run_bass_kernel_spmd(nc, in_maps, core_ids=list(range(8)))
    y = np.zeros((B, S, D), dtype=np.float32)
    for c in range(8):
        y[c // 4] += res.results[c]["out"].astype(np.float32)
    return y


# revision 29
# speedup vs baseline: 1.0591x; 1.0591x over previous
"""GQA attention (RoPE, causal) + o_proj on 8 Trainium2 NeuronCores.

Sharding: 8 cores = 2 batches (DP) x 4 kv-head-pairs (TP over GQA groups).
Per core: hsT[batch] [D,S] (host-pretransposed), Wq slice (8 q heads), Wk/Wv
slice (2 kv heads), Wo slice [512,D]. Core computes its heads' attention and a
partial o_proj output [S,D] (fp16); host sums 4 partials per batch in fp32.

v2 kernel dataflow (per core; fp16 matmul operands, fp32 PSUM accumulate),
J-pipelined over 4 seq-blocks of 512:
  1. qT/kT produced DIRECTLY transposed: stationary W-tile [128D x 128f],
     streaming hsT [128D x 512s] -> psum [f, s]. No PE transposes for q/k.
  2. RoPE in transposed layout: host pair-interleaves each head's features
     (p=2i <-> x_i, p=2i+1 <-> x_{i+32}) so rotate-half == partition pair-swap
     == DVE stream_shuffle(mask=[1,0,3,2,...]). Tables are per-partition
     [128, S] fp16, sign-folded, q-scaled.
  3. v transposed back to natural via 4 PE transposes per J-block -> vaug
     [seq, 65] with ones column for the softmax denominator.
  4. scoresT[k,q] per 128k-tile x 512q block, 2 heads row-packed via
     tile_position (0,0)/(64,0); causal k-tiles only; 2 k-tiles per PSUM
     [128,1024]; exp on ACT -> fp16 pt.
  5. causal mask: gpsimd affine_select zeroes the [128,128] triangle on
     diagonal tiles only; fully-masked column blocks are simply excluded
     from the AV streams (column-restricted accumulation).
  6. AT_aug = [V|1].T @ P^T accumulated over k-tiles in PSUM; row 64 is the
     denominator; normalize via reciprocal + gpsimd partition_broadcast.
  7. o_proj(J-1) quarters interleaved into attention(J) as PE filler;
     y emitted fp16.
"""
import sys
import numpy as np

sys.path.insert(0, "/opt/trn_rl_repo")

B, S, D = 2, 2048, 2048
H, KVH, HD = 32, 8, 64
SCALE = HD ** -0.5
P = 128


def build_nc(S=S, D=D, LQ=8, LKV=2, HD=64):
    import concourse.bacc as bacc
    import concourse.mybir as mybir
    from concourse import tile
    from concourse.masks import make_identity

    f32 = mybir.dt.float32
    f16 = mybir.dt.float16

    QF = LQ * HD          # local q features (512)
    KF = LKV * HD         # local kv features (128)
    FT = QF // P          # q feature tiles = head pairs (4)
    DT = D // P           # contraction tiles (16)
    ST = S // P           # sequence k-tiles (16)
    NJ = S // 512         # seq super-blocks (4)
    SB = 512              # seq block size
    VW = HD + 1           # v + ones column (65)
    Exp = mybir.ActivationFunctionType.Exp
    NEG = -30000.0

    nc = bacc.Bacc(None, target_bir_lowering=False)
    hsT = nc.declare_dram_parameter("hsT", [D, S], f16, isOutput=False)
    wq = nc.declare_dram_parameter("wq", [D, QF], f16, isOutput=False)
    wk = nc.declare_dram_parameter("wk", [D, KF], f16, isOutput=False)
    wv = nc.declare_dram_parameter("wv", [D, KF], f16, isOutput=False)
    wo = nc.declare_dram_parameter("wo", [QF, D], f16, isOutput=False)
    cq = nc.declare_dram_parameter("cq", [P, S], f16, isOutput=False)
    sq = nc.declare_dram_parameter("sq", [P, S], f16, isOutput=False)
    ck = nc.declare_dram_parameter("ck", [P, S], f16, isOutput=False)
    sk = nc.declare_dram_parameter("sk", [P, S], f16, isOutput=False)
    out = nc.declare_dram_parameter("out", [S, D], f16, isOutput=True)

    hsT_r = hsT.rearrange("(dt p) s -> p dt s", p=P)
    wq_r = wq.rearrange("(dt p) f -> p dt f", p=P)
    wk_r = wk.rearrange("(dt p) f -> p dt f", p=P)
    wv_r = wv.rearrange("(dt p) f -> p dt f", p=P)
    wo_r = wo.rearrange("(ft p) d -> p ft d", p=P)
    out_r = out.rearrange("(st p) d -> p st d", p=P)

    # stream_shuffle mask: swap adjacent pairs within each 32-lane quadrant
    pairswap = []
    for i in range(16):
        pairswap += [2 * i + 1, 2 * i]

    with tile.TileContext(nc) as tc:
        with tc.tile_pool(name="persist", bufs=1) as persist:
            ident = persist.tile([P, P], f16)
            tri = persist.tile([P, P], f16)
            k2T = persist.tile([P, LKV, S], f16)
            vaug = persist.tile([P, ST, LKV * VW], f16)
            cq_sb = persist.tile([P, S], f16)
            sq_sb = persist.tile([P, S], f16)
            ck_sb = persist.tile([P, S], f16)
            sk_sb = persist.tile([P, S], f16)
            wq_sb = persist.tile([P, DT, QF], f16)
            wk_sb = persist.tile([P, DT, KF], f16)
            wv_sb = persist.tile([P, DT, KF], f16)
            wo_sb = persist.tile([P, FT, D], f16)

            with tc.tile_pool(name="init", bufs=1) as init_p:
                ident_f = init_p.tile([P, P], f32)
                make_identity(nc, ident_f[:])
                nc.vector.tensor_copy(ident[:], ident_f[:])
                # tri[kr, j] = 1 where j >= kr else 0 (causal triangle keep-mask)
                tri_f = init_p.tile([P, P], f32)
                nc.gpsimd.memset(tri_f[:], 1.0)
                nc.gpsimd.affine_select(
                    out=tri_f[:], in_=tri_f[:],
                    compare_op=mybir.AluOpType.is_ge, fill=0.0,
                    base=0, pattern=[[1, P]], channel_multiplier=-1)
                nc.vector.tensor_copy(tri[:], tri_f[:])
                # ones columns of vaug (col 64 and 129 of the 130-wide dim)
                for lkv in range(LKV):
                    nc.gpsimd.memset(vaug[:, :, lkv * VW + HD:lkv * VW + VW], 1.0)

            with (
                tc.tile_pool(name="hs_p", bufs=2) as hs_p,
                tc.tile_pool(name="rope_p", bufs=3) as rope_p,
                tc.tile_pool(name="qT_p", bufs=2) as qT_p,
                tc.tile_pool(name="aT_p", bufs=2) as aT_p,
                tc.tile_pool(name="pt_p", bufs=18) as pt_p,
                tc.tile_pool(name="y_p", bufs=4) as y_p,
                tc.tile_pool(name="rd_p", bufs=4) as rd_p,
                tc.tile_pool(name="bc_p", bufs=4) as bc_p,
                tc.tile_pool(name="pmix", bufs=2, space="PSUM") as pmix,
                tc.tile_pool(name="ps_s", bufs=2, space="PSUM") as ps_s,
                tc.tile_pool(name="ps_a", bufs=2, space="PSUM") as ps_a,
            ):
                # DMA order matters: the first projection chain needs hsT(J0)
                # in dt order and wq's first feature tile; tables shortly
                # after; wo last. hs is split so the first chain can start
                # before the whole block lands.
                hsb_first = hs_p.tile([P, DT, SB], f16, tag="hsb")
                wq_r4 = wq.rearrange("(dt p) (ft f) -> p ft dt f", p=P, f=P)
                wq_sb4 = wq_sb[:].rearrange("p dt (ft f) -> p ft dt f", f=P)
                # single HWDGE stream ordered by first consumption
                nc.sync.dma_start(
                    out=hsb_first[:, 0:4], in_=hsT_r[:, 0:4, 0:SB])
                nc.sync.dma_start(out=wq_sb4[:, 0], in_=wq_r4[:, 0])
                nc.sync.dma_start(
                    out=hsb_first[:, 4:16], in_=hsT_r[:, 4:16, 0:SB])
                nc.sync.dma_start(out=cq_sb[:], in_=cq[:, :])
                nc.sync.dma_start(out=sq_sb[:], in_=sq[:, :])
                nc.sync.dma_start(out=wq_sb4[:, 1], in_=wq_r4[:, 1])
                nc.sync.dma_start(out=wq_sb4[:, 2], in_=wq_r4[:, 2])
                nc.sync.dma_start(out=wq_sb4[:, 3], in_=wq_r4[:, 3])
                nc.sync.dma_start(out=wk_sb[:], in_=wk_r)
                nc.sync.dma_start(out=ck_sb[:], in_=ck[:, :])
                nc.sync.dma_start(out=sk_sb[:], in_=sk[:, :])
                nc.sync.dma_start(out=wv_sb[:], in_=wv_r)
                hsb_second = hs_p.tile([P, DT, SB], f16, tag="hsb")
                nc.sync.dma_start(out=hsb_second[:], in_=hsT_r[:, :, SB:2 * SB])
                nc.sync.dma_start(out=wo_sb[:], in_=wo_r)
                def rope(ps, cos_sb, sin_sb, j0, dest):
                    """dest = ps * cos + pairswap(ps * sin_pre)  [P, SB] fp16.

                    sin_pre is the host-pre-shuffled (and sign-folded) sin
                    table, so the pair-swap happens AFTER the multiply and
                    stream_shuffle runs same-dtype fp16->fp16.
                    """
                    u = rope_p.tile([P, SB], f16, tag="u")
                    nc.vector.tensor_mul(
                        u[:], ps[:], sin_sb[:, j0:j0 + SB])
                    tmp = rope_p.tile([P, SB], f16, tag="tmp")
                    nc.vector.tensor_mul(
                        tmp[:], ps[:], cos_sb[:, j0:j0 + SB])
                    sh = rope_p.tile([P, SB], f16, tag="sh")
                    nc.vector.stream_shuffle(sh[:], u[:], mask=pairswap)
                    nc.vector.tensor_add(dest, tmp[:], sh[:])

                def o_proj_quarter(aTt, Jp, stl):
                    st = 4 * Jp + stl
                    yt = y_p.tile([P, D], f16, tag="yt")
                    for dn_i in range(D // 512):
                        psy = pmix.tile([P, 512], f32, tag="ps512")
                        for ft in range(FT):
                            nc.tensor.matmul(
                                psy[:],
                                aTt[:, ft, stl * P:(stl + 1) * P],
                                wo_sb[:, ft, dn_i * 512:(dn_i + 1) * 512],
                                start=(ft == 0), stop=(ft == FT - 1))
                        nc.any.tensor_copy(
                            yt[:, dn_i * 512:(dn_i + 1) * 512], psy[:])
                    nc.sync.dma_start(out=out_r[:, st, :], in_=yt[:])

                def proj(J, hsb):
                    """projections for seq block J -> (qT tile, k2T/vaug)."""
                    j0 = J * SB
                    qTt = qT_p.tile([P, FT, SB], f16, tag="qT")
                    for ft in range(FT):
                        psq = pmix.tile([P, SB], f32, tag="ps512")
                        for dt in range(DT):
                            nc.tensor.matmul(
                                psq[:],
                                wq_sb[:, dt, ft * P:(ft + 1) * P],
                                hsb[:, dt, :],
                                start=(dt == 0), stop=(dt == DT - 1))
                        rope(psq, cq_sb, sq_sb, j0, qTt[:, ft, :])

                    psk = pmix.tile([P, SB], f32, tag="ps512")
                    for dt in range(DT):
                        nc.tensor.matmul(
                            psk[:], wk_sb[:, dt, :], hsb[:, dt, :],
                            start=(dt == 0), stop=(dt == DT - 1))
                    krot = rope_p.tile([P, SB], f16, tag="krot")
                    rope(psk, ck_sb, sk_sb, j0, krot[:])
                    nc.vector.tensor_copy(
                        k2T[0:HD, 0, j0:j0 + SB], krot[0:HD, :])
                    nc.vector.tensor_copy(
                        k2T[HD:P, 0, j0:j0 + SB], krot[0:HD, :])
                    nc.vector.tensor_copy(
                        k2T[0:HD, 1, j0:j0 + SB], krot[HD:P, :])
                    nc.vector.tensor_copy(
                        k2T[HD:P, 1, j0:j0 + SB], krot[HD:P, :])

                    # v natural-direct (stationary hs block) -> vaug
                    psv = pmix.tile([P, SB], f32, tag="ps512")
                    for si in range(4):
                        for dt in range(DT):
                            nc.tensor.matmul(
                                psv[:, si * P:(si + 1) * P],
                                hsb[:, dt, si * P:(si + 1) * P],
                                wv_sb[:, dt, :],
                                start=(dt == 0), stop=(dt == DT - 1))
                    psv_v = psv[:].rearrange("p (q f) -> p q f", q=4)
                    nc.vector.tensor_copy(
                        vaug[:, 4 * J:4 * J + 4, 0:HD], psv_v[:, :, 0:HD])
                    nc.vector.tensor_copy(
                        vaug[:, 4 * J:4 * J + 4, VW:VW + HD], psv_v[:, :, HD:P])
                    return qTt

                # macro-order: proj0 proj1 attn0 proj2 attn1+op0 proj3
                # attn2+op1 attn3+op2 op3 -- every attention phase has
                # projection or o_proj matmuls available as PE filler
                qTs = {0: proj(0, hsb_first), 1: proj(1, hsb_second)}
                hsbs = {}
                prev_aT = None
                for J in range(NJ):
                    j0 = J * SB
                    qTt = qTs.pop(J)
                    if J + 2 < NJ:
                        hsb_pf = hs_p.tile([P, DT, SB], f16, tag="hsb")
                        hsbs[J + 2] = hsb_pf
                        nc.sync.dma_start(
                            out=hsb_pf[:],
                            in_=hsT_r[:, :, (J + 2) * SB:(J + 3) * SB])

                    # ---- attention for this J block ----
                    nkt = 4 * J + 4
                    aTt = aT_p.tile([P, FT, SB], f16, tag="aT")
                    for t in range(FT):          # head pair (2t, 2t+1)
                        lkv = t // 2
                        # scores: both heads of the pair share one PSUM tile
                        # (cols 0:512 head A, 512:1024 head B) so the two
                        # row-group matmuls become ready together and stay
                        # adjacent -> true tile_position concurrency.
                        pts = []
                        for kt in range(nkt):
                            pss = ps_s.tile([P, 1024], f32, tag="pss")
                            nc.tensor.matmul(
                                pss[:, 0:512],
                                k2T[0:HD, lkv, kt * P:(kt + 1) * P],
                                qTt[0:HD, t, :],
                                start=True, stop=True,
                                tile_position=(0, 0))
                            nc.tensor.matmul(
                                pss[:, 512:1024],
                                k2T[HD:P, lkv, kt * P:(kt + 1) * P],
                                qTt[HD:P, t, :],
                                start=True, stop=True,
                                tile_position=(HD, 0))
                            pt = pt_p.tile([P, 1024], f16, tag="pt")
                            m = kt - 4 * J
                            if m >= 1:
                                # diagonal tile: columns < 128m are fully
                                # masked; exp only the visible column ranges
                                # of both heads via a strided AP
                                pss_v = pss[:].rearrange(
                                    "p (h c) -> p h c", h=2)
                                pt_v = pt[:].rearrange("p (h c) -> p h c", h=2)
                                nc.scalar.activation(
                                    pt_v[:, :, m * P:512],
                                    pss_v[:, :, m * P:512], Exp)
                            else:
                                nc.scalar.activation(pt[:], pss[:], Exp)
                            if m >= 0:               # diagonal tile: triangle
                                nc.vector.tensor_mul(
                                    pt[:, m * P:(m + 1) * P],
                                    pt[:, m * P:(m + 1) * P], tri[:])
                                nc.vector.tensor_mul(
                                    pt[:, 512 + m * P:512 + (m + 1) * P],
                                    pt[:, 512 + m * P:512 + (m + 1) * P],
                                    tri[:])
                            pts.append(pt)
                        psa0 = ps_a.tile([VW, 512], f32, tag="psa")
                        psa1 = ps_a.tile([VW, 512], f32, tag="psa")
                        for kt in range(nkt):
                            pt = pts[kt]
                            c0 = (kt - 4 * J) * P if kt >= 4 * J else 0
                            nc.tensor.matmul(
                                psa0[:, c0:512],
                                vaug[:, kt, lkv * VW:(lkv + 1) * VW],
                                pt[:, c0:512],
                                start=(kt == 0), stop=(kt == nkt - 1),
                                skip_group_check=True)
                            nc.tensor.matmul(
                                psa1[:, c0:512],
                                vaug[:, kt, lkv * VW:(lkv + 1) * VW],
                                pt[:, 512 + c0:1024],
                                start=(kt == 0), stop=(kt == nkt - 1),
                                skip_group_check=True)
                        # normalize: psa rows 0..63 / row 64
                        for psa, poff in ((psa0, 0), (psa1, HD)):
                            dn = rd_p.tile([1, 512], f32, tag="dn")
                            nc.vector.tensor_copy(dn[:], psa[HD:VW, :])
                            rc = rd_p.tile([1, 512], f32, tag="rc")
                            nc.vector.reciprocal_approx_fast(rc[:], dn[:])
                            dnb = bc_p.tile([HD, 512], f32, tag="bc")
                            nc.gpsimd.partition_broadcast(dnb[:], rc[:])
                            nc.vector.tensor_mul(
                                aTt[poff:poff + HD, t, :], psa[0:HD, :], dnb[:])
                        # PE filler while exp/normalize run
                        if prev_aT is not None:
                            o_proj_quarter(prev_aT, J - 1, t)
                    prev_aT = aTt
                    if J + 2 < NJ:
                        qTs[J + 2] = proj(J + 2, hsbs.pop(J + 2))
                for stl in range(4):
                    o_proj_quarter(prev_aT, NJ - 1, stl)
    nc.compile()
    return nc


def _host_tables(LQ, LKV, scale):
    """Per-partition [128, S] fp16 tables in pair-interleaved feature order.

    partition p (within a 64-feature head): freq index i = p//2;
    p even holds x_i   -> rot term is -x_{i+32} * sin -> sin sign -1
    p odd  holds x_{i+32} -> rot term is +x_i * sin   -> sin sign +1
    """
    hd = HD
    inv_freq = 1.0 / (10000.0 ** (np.arange(0, hd, 2, dtype=np.float64) / hd))
    t = np.arange(S, dtype=np.float64)
    freqs = np.outer(inv_freq, t)                    # [32, S]
    cos64 = np.repeat(np.cos(freqs), 2, axis=0)      # [64, S]
    sin64 = np.repeat(np.sin(freqs), 2, axis=0)
    sgn = np.where(np.arange(64) % 2 == 0, -1.0, 1.0)[:, None]
    sin64 = sin64 * sgn
    # pre-shuffle rows by the pair-swap so the kernel can shuffle AFTER
    # multiplying (keeps stream_shuffle same-dtype fp16->fp16)
    rowswap = np.arange(64) ^ 1
    sin64 = sin64[rowswap]
    cosP = np.tile(cos64, (2, 1))                    # [128, S]
    sinP = np.tile(sin64, (2, 1))
    cq = (cosP * scale).astype(np.float16)
    sq = (sinP * scale).astype(np.float16)
    ck = cosP.astype(np.float16)
    sk = sinP.astype(np.float16)
    return cq, sq, ck, sk


def _perm64():
    """pair-interleave: new position 2i <- old i, 2i+1 <- old 32+i."""
    p = np.empty(64, dtype=np.int64)
    p[0::2] = np.arange(32)
    p[1::2] = np.arange(32) + 32
    return p


def prepare_in_maps(hidden_states, cos, sin, Wq, Wk, Wv, Wo, LQ=8, LKV=2):
    cq, sq, ck, sk = _host_tables(LQ, LKV, SCALE)
    nb = hidden_states.shape[0]
    hsT = [np.ascontiguousarray(hidden_states[b].T).astype(np.float16)
           for b in range(nb)]
    p64 = _perm64()
    permQ = np.concatenate([p64 + 64 * h for h in range(LQ)])
    permK = np.concatenate([p64 + 64 * h for h in range(LKV)])
    in_maps = []
    for c in range(8):
        b, g2 = c // 4, c % 4
        qs = g2 * LQ * HD
        ks = g2 * LKV * HD
        wq_l = Wq[:, qs:qs + LQ * HD][:, permQ]
        wk_l = Wk[:, ks:ks + LKV * HD][:, permK]
        wv_l = Wv[:, ks:ks + LKV * HD]
        in_maps.append({
            "hsT": hsT[b],
            "wq": np.ascontiguousarray(wq_l).astype(np.float16),
            "wk": np.ascontiguousarray(wk_l).astype(np.float16),
            "wv": np.ascontiguousarray(wv_l).astype(np.float16),
            "wo": np.ascontiguousarray(Wo[qs:qs + LQ * HD, :]).astype(np.float16),
            "cq": cq, "sq": sq, "ck": ck, "sk": sk,
        })
    return in_maps


_NC_CACHE = {}


def kernel(hidden_states, attention_mask, cos, sin, Wq, Wk, Wv, Wo):
    from concourse.bass_utils import run_bass_kernel_spmd

    hidden_states = np.asarray(hidden_states, dtype=np.float32)
    cos = np.asarray(cos, dtype=np.float32)
    sin = np.asarray(sin, dtype=np.float32)
    Wq = np.asarray(Wq, dtype=np.float32)
    Wk = np.asarray(Wk, dtype=np.float32)
    Wv = np.asarray(Wv, dtype=np.float32)
    Wo = np.asarray(Wo, dtype=np.float32)

    LQ, LKV = 8, 2
    if "nc" not in _NC_CACHE:
        _NC_CACHE["nc"] = build_nc(S, D, LQ, LKV, HD)
    nc = _NC_CACHE["nc"]

    in_maps = prepare_in_maps(hidden_states, cos, sin, Wq, Wk, Wv, Wo, LQ, LKV)
    res = run_bass_kernel_spmd(nc, in_maps, core_ids=list(range(8)))
    y = np.zeros((B, S, D), dtype=np.float32)
    for c in range(8):
        y[c // 4] += res.results[c]["out"].astype(np.float32)
    return y
